# revision 5
# baseline (speedup 1.0000x reference)
"""Distributed Trainium2 kernel for the attention GEMV chain:

    score = context_vector @ query            [L]         (L=8192, Q=4096)
    attn  = softmax(score)
    s_t   = attn @ context_vector             [Q]
    out   = K_w @ concat(query, s_t)          [Q]

Sharding over 8 NeuronCores:
  - context_vector rows: 1024 per core (score GEMV + partial weighted sums)
  - K_w rows: 512 per core (each core produces its own slice of the output,
    so no output collective is needed)
  - one AllGather of 8 per-tile rows [s_t_partial(4096), tile_max, tile_expsum]
    per core; the softmax normalization is finished after the gather by an
    alpha-weighted rank-64 matmul that also broadcasts s_t to 128 partitions.

Per-core schedule (flash-style, everything overlapped with the DMA stream):
  - per 128-row tile of cv: fused mult+reduce (scalar_tensor_tensor) gives
    scores; gpsimd partition_all_reduce gives the tile max; ACT exp gives the
    tile weights; TensorE matmuls the exp-weighted row sum into a PSUM row
    (partitions cycle {0,32,64}); ACT copies the row to SBUF; DMA stages it
    for the collective. The cv tile is then free, so the K_w stream starts
    early.
  - K_w is loaded as 8 half-tiles [128, 4096]; the query-half dot products
    run before/during the collective, the s_t-half ones read the broadcast
    s_t directly from PSUM.
"""
import sys

if "/opt/trn_rl_repo" not in sys.path:
    sys.path.insert(0, "/opt/trn_rl_repo")

from contextlib import ExitStack

import numpy as np

import concourse.bass as bass
import concourse.bacc as bacc
import concourse.mybir as mybir
import concourse.tile as tile
from concourse.bass_isa import ReduceOp
from concourse.bass_utils import run_bass_kernel_spmd

N_CORES = 8
Q = 4096
L = 8192
L_SHARD = L // N_CORES          # 1024 rows of context_vector per core
R_SHARD = Q // N_CORES          # 512 rows of K_w per core
LT = L_SHARD // 128             # 8 l-tiles per core
RT = R_SHARD // 128             # 4 r-tiles per core
NB = Q // 512                   # 8 psum banks of 512 fp32
CCW = Q + 8                     # collective row: partial(4096), max, sum, pad
GROWS = N_CORES * LT            # 64 gathered rows
DT = mybir.dt.float32

_NC_CACHE = {}


def build_nc():
    nc = bacc.Bacc("TRN2", target_bir_lowering=False, debug=False,
                   num_devices=N_CORES)

    q_ext = nc.dram_tensor("query", [1, Q], DT, kind="ExternalInput")
    cv_ext = nc.dram_tensor("cv", [L_SHARD, Q], DT, kind="ExternalInput")
    kw_ext = nc.dram_tensor("kw", [R_SHARD, 2 * Q], DT, kind="ExternalInput")
    out_ext = nc.dram_tensor("out", [128, RT], DT, kind="ExternalOutput")

    cc_in = nc.dram_tensor("cc_in", [1, LT * CCW], DT)
    cc_out = nc.dram_tensor("cc_out", [N_CORES, LT * CCW], DT,
                            addr_space="Shared")

    with tile.TileContext(nc) as tc, ExitStack() as ctx:
        persist = ctx.enter_context(tc.tile_pool(name="persist", bufs=1))
        smalls = ctx.enter_context(tc.tile_pool(name="smalls", bufs=1))

        # query broadcast to all 128 partitions via stride-0 DMA
        queryB = persist.tile([128, Q], DT)
        qa = q_ext.ap()
        q_bcast = bass.AP(tensor=qa.tensor, offset=qa.offset,
                          ap=[[0, 128], list(qa.ap[-1])])
        nc.sync.dma_start(out=queryB, in_=q_bcast)

        scores = smalls.tile([128, LT], DT)
        dummy = smalls.tile([128, 1], DT)
        mstack = smalls.tile([128, LT], DT)     # per-tile max (replicated)
        nstack = smalls.tile([128, LT], DT)     # negated maxes
        estack = smalls.tile([128, LT], DT)     # per-tile exp weights
        sstack = smalls.tile([128, LT], DT)     # per-tile expsum (replicated)
        stage = persist.tile([128, Q], DT)      # staged s_t rows {0,32,64}
        ones_rep = smalls.tile([GROWS, 128], DT)
        nc.vector.memset(ones_rep, 1.0)

        # ---- phase 1: stream cv; per-tile scores, stats, weighted row ----
        with tc.tile_pool(name="cvp", bufs=3) as cvp, \
             tc.tile_pool(name="ps1", bufs=1, space="PSUM") as ps1:
            psum_st = ps1.tile([128, Q], DT)
            for t in range(LT):
                r = 32 * (t % 3)
                cv_t = cvp.tile([128, Q], DT)
                nc.sync.dma_start(out=cv_t,
                                  in_=cv_ext[t * 128:(t + 1) * 128, :])
                nc.vector.scalar_tensor_tensor(
                    out=dummy.broadcast_to([128, Q]),
                    in0=cv_t, scalar=1.0, in1=queryB,
                    op0=mybir.AluOpType.mult, op1=mybir.AluOpType.mult,
                    accum_out=scores[:, t:t + 1],
                )
                nc.gpsimd.partition_all_reduce(
                    mstack[:, t:t + 1], scores[:, t:t + 1], 128, ReduceOp.max)
                nc.gpsimd.tensor_scalar_mul(
                    nstack[:, t:t + 1], mstack[:, t:t + 1], -1.0)
                nc.scalar.activation(
                    out=estack[:, t:t + 1], in_=scores[:, t:t + 1],
                    func=mybir.ActivationFunctionType.Exp,
                    bias=nstack[:, t:t + 1], scale=1.0)
                nc.gpsimd.partition_all_reduce(
                    sstack[:, t:t + 1], estack[:, t:t + 1], 128, ReduceOp.add)
                for n in range(NB):
                    sl = slice(n * 512, (n + 1) * 512)
                    nc.tensor.matmul(
                        psum_st[r:r + 1, sl],
                        lhsT=estack[:, t:t + 1],
                        rhs=cv_t[:, sl],
                        start=True, stop=True,
                    )
                # stage the finished row and ship it to the collective buffer
                nc.scalar.copy(stage[r:r + 1, :], psum_st[r:r + 1, :])
                row_out = bass.AP(tensor=cc_in.ap().tensor, offset=t * CCW,
                                  ap=[[0, 1], [1, Q]])
                nc.sync.dma_start(out=row_out, in_=stage[r:r + 1, :])

        # per-tile stats into the collective rows
        m_out = bass.AP(tensor=cc_in.ap().tensor, offset=Q,
                        ap=[[0, 1], [CCW, LT]])
        nc.sync.dma_start(out=m_out, in_=mstack[0:1, 0:LT])
        s_out = bass.AP(tensor=cc_in.ap().tensor, offset=Q + 1,
                        ap=[[0, 1], [CCW, LT]])
        nc.sync.dma_start(out=s_out, in_=sstack[0:1, 0:LT])

        # ---- phase 2: K_w query-half dots (overlap the collective) ----
        accq = smalls.tile([128, RT], DT)
        accs = smalls.tile([128, RT], DT)
        acc = smalls.tile([128, RT], DT)
        with tc.tile_pool(name="kwq", bufs=2) as kwq:
            for j in range(RT):
                kwq_j = kwq.tile([128, Q], DT)
                nc.sync.dma_start(out=kwq_j,
                                  in_=kw_ext[j * 128:(j + 1) * 128, 0:Q])
                nc.vector.scalar_tensor_tensor(
                    out=dummy.broadcast_to([128, Q]),
                    in0=kwq_j, scalar=1.0, in1=queryB,
                    op0=mybir.AluOpType.mult, op1=mybir.AluOpType.mult,
                    accum_out=accq[:, j:j + 1],
                )

        # ---- phase 3: AllGather the 64 rows ----
        nc.gpsimd.collective_compute(
            "AllGather",
            mybir.AluOpType.bypass,
            replica_groups=[list(range(N_CORES))],
            ins=[cc_in.ap().opt()],
            outs=[cc_out.ap().opt()],
        )
        gathered = persist.tile([GROWS, CCW], DT)
        gin = bass.AP(tensor=cc_out.ap().tensor, offset=0,
                      ap=[[CCW, GROWS], [1, CCW]])
        nc.scalar.dma_start(out=gathered, in_=gin)

        # ---- phase 4: global softmax combine, s_t broadcast into PSUM ----
        mg = gathered[:, Q:Q + 1]
        sg = gathered[:, Q + 1:Q + 2]
        mmax = smalls.tile([GROWS, 1], DT)
        nc.gpsimd.partition_all_reduce(mmax, mg, GROWS, ReduceOp.max)
        negM = smalls.tile([GROWS, 1], DT)
        nc.gpsimd.tensor_scalar_mul(negM, mmax, -1.0)
        expm = smalls.tile([GROWS, 1], DT)
        nc.scalar.activation(out=expm, in_=mg,
                             func=mybir.ActivationFunctionType.Exp,
                             bias=negM, scale=1.0)
        w = smalls.tile([GROWS, 1], DT)
        nc.vector.tensor_mul(w, expm, sg)
        wsum = smalls.tile([GROWS, 1], DT)
        nc.gpsimd.partition_all_reduce(wsum, w, GROWS, ReduceOp.add)
        rS = smalls.tile([GROWS, 1], DT)
        nc.vector.reciprocal(rS, wsum)
        alpha = smalls.tile([GROWS, 1], DT)
        nc.vector.tensor_mul(alpha, expm, rS)
        alpha_rep = smalls.tile([GROWS, 128], DT)
        nc.vector.tensor_scalar_mul(alpha_rep, ones_rep, alpha)

        with tc.tile_pool(name="ps2", bufs=1, space="PSUM") as ps2, \
             tc.tile_pool(name="kws", bufs=3) as kws:
            psum_stB = ps2.tile([128, Q], DT)
            for n in range(NB):
                sl = slice(n * 512, (n + 1) * 512)
                nc.tensor.matmul(
                    psum_stB[:, sl],
                    lhsT=alpha_rep,
                    rhs=gathered[0:GROWS, sl],
                    start=True, stop=True,
                )

            # ---- phase 5: K_w s_t-half dots against PSUM-resident s_t ----
            for j in range(RT):
                kws_j = kws.tile([128, Q], DT)
                nc.sync.dma_start(out=kws_j,
                                  in_=kw_ext[j * 128:(j + 1) * 128, Q:2 * Q])
                nc.vector.scalar_tensor_tensor(
                    out=dummy.broadcast_to([128, Q]),
                    in0=kws_j, scalar=1.0, in1=psum_stB,
                    op0=mybir.AluOpType.mult, op1=mybir.AluOpType.mult,
                    accum_out=accs[:, j:j + 1],
                )

        nc.vector.tensor_add(acc, accq, accs)
        nc.sync.dma_start(out=out_ext.ap(), in_=acc)

    nc.compile()
    return nc


def get_nc():
    if "nc" not in _NC_CACHE:
        _NC_CACHE["nc"] = build_nc()
    return _NC_CACHE["nc"]


def _shard_inputs(query, context_vector, K_w):
    q2 = np.ascontiguousarray(query.reshape(1, Q), dtype=np.float32)
    in_maps = []
    for c in range(N_CORES):
        in_maps.append({
            "query": q2,
            "cv": np.ascontiguousarray(
                context_vector[c * L_SHARD:(c + 1) * L_SHARD], dtype=np.float32),
            "kw": np.ascontiguousarray(
                K_w[c * R_SHARD:(c + 1) * R_SHARD], dtype=np.float32),
        })
    return in_maps


def kernel(query, context_vector, K_w, _trace=False, _trace_kwargs=None):
    nc = get_nc()
    in_maps = _shard_inputs(query, context_vector, K_w)
    res = run_bass_kernel_spmd(nc, in_maps, core_ids=list(range(N_CORES)),
                               trace=_trace, **(_trace_kwargs or {}))
    out = np.concatenate(
        [np.asarray(res.results[c]["out"]).T.reshape(-1) for c in range(N_CORES)]
    ).astype(np.float32)
    if _trace:
        kernel.last_results = res
    return out


# revision 10
# speedup vs baseline: 1.2180x; 1.2180x over previous
"""Distributed Trainium2 kernel for the attention GEMV chain:

    score = context_vector @ query            [L]         (L=8192, Q=4096)
    attn  = softmax(score)
    s_t   = attn @ context_vector             [Q]
    out   = K_w @ concat(query, s_t)          [Q]

Sharding over 8 NeuronCores:
  - context_vector rows: 1024 per core (score GEMV + partial weighted sums)
  - K_w rows: 512 per core (each core produces its own slice of the final
    output, so no output collective is needed)
  - one AllGather of 8 per-tile rows [s_t_partial(4096), tile_max, expsum]
    per core (64 rows total); the softmax normalization finishes after the
    gather with an alpha-weighted rank-64 bf16 matmul that also broadcasts
    s_t to all 128 partitions (in PSUM).

Per-core schedule (flash-style, paced by the DMA stream):
  - per 128-row cv tile: fused mult+reduce (scalar_tensor_tensor) gives the
    128 scores in one DVE pass; gpsimd partition_all_reduce gives the tile
    max (per-tile reference keeps exp <= 1, always fp32-safe); ACT computes
    bf16 exp weights and casts the tile to bf16 (cast alternates DVE/ACT to
    balance engine load); TensorE matmuls the exp-weighted row sum into a
    PSUM row cycling partitions {0,32,64}; ACT copies the row out and it is
    DMA-staged for the collective immediately — so the cv tile frees early
    and the K_w stream starts long before the softmax is complete.
  - scores stay fp32 end-to-end: the softmax is argmax-dominated and exp is
    intolerant of score error; the weights and matrices tolerate bf16.
  - K_w streams as 8 half-tiles [128, 4096]; the query-half dot products
    run before/during the collective, the s_t-half ones read the broadcast
    s_t directly from PSUM.
"""
import sys

if "/opt/trn_rl_repo" not in sys.path:
    sys.path.insert(0, "/opt/trn_rl_repo")

from contextlib import ExitStack

import numpy as np

import concourse.bass as bass
import concourse.bacc as bacc
import concourse.mybir as mybir
import concourse.tile as tile
from concourse.bass_isa import ReduceOp
from concourse.bass_utils import run_bass_kernel_spmd

N_CORES = 8
Q = 4096
L = 8192
L_SHARD = L // N_CORES          # 1024 rows of context_vector per core
R_SHARD = Q // N_CORES          # 512 rows of K_w per core
LT = L_SHARD // 128             # 8 l-tiles per core
RT = R_SHARD // 128             # 4 r-tiles per core
NB = Q // 512                   # 8 psum banks of 512 fp32
CCW = Q + 8                     # collective row: partial(4096), max, sum, pad
GROWS = N_CORES * LT            # 64 gathered rows
DT = mybir.dt.float32
BF = mybir.dt.bfloat16

_NC_CACHE = {}
_DEBUG = False


def build_nc():
    nc = bacc.Bacc("TRN2", target_bir_lowering=False, debug=False,
                   num_devices=N_CORES)

    q_ext = nc.dram_tensor("query", [1, Q], DT, kind="ExternalInput")
    cv_ext = nc.dram_tensor("cv", [L_SHARD, Q], DT, kind="ExternalInput")
    kw_ext = nc.dram_tensor("kw", [R_SHARD, 2 * Q], DT, kind="ExternalInput")
    out_ext = nc.dram_tensor("out", [128, RT], DT, kind="ExternalOutput")

    cc_in = nc.dram_tensor("cc_in", [1, LT * CCW], DT)
    cc_out = nc.dram_tensor("cc_out", [N_CORES, LT * CCW], DT,
                            addr_space="Shared")
    dbg_ext = None
    if _DEBUG:
        dbg_ext = nc.dram_tensor("dbg", [16, 16], DT, kind="ExternalOutput")

    with tile.TileContext(nc) as tc, ExitStack() as ctx:
        persist = ctx.enter_context(tc.tile_pool(name="persist", bufs=1))
        smalls = ctx.enter_context(tc.tile_pool(name="smalls", bufs=1))

        # query broadcast to all 128 partitions via stride-0 DMA
        queryB = persist.tile([128, Q], DT)
        qa = q_ext.ap()
        q_bcast = bass.AP(tensor=qa.tensor, offset=qa.offset,
                          ap=[[0, 128], list(qa.ap[-1])])
        nc.sync.dma_start(out=queryB, in_=q_bcast)

        scores = smalls.tile([128, LT], DT)
        dummy = smalls.tile([128, 1], DT)
        mstack = smalls.tile([128, LT], DT)     # per-tile max (replicated)
        nstack = smalls.tile([128, LT], DT)     # negated maxes
        estack = smalls.tile([128, LT], BF)     # per-tile bf16 exp weights
        sstack = smalls.tile([128, LT], DT)     # per-tile expsum (replicated)
        stage = persist.tile([128, Q], DT)      # staged rows at {0,32,64}
        ones_rep = smalls.tile([GROWS, 128], BF)
        nc.vector.memset(ones_rep, 1.0)

        # ---- phase 1: stream cv; per-tile scores, stats, weighted row ----
        with tc.tile_pool(name="cvp", bufs=2) as cvp, \
             tc.tile_pool(name="cvb", bufs=2) as cvb, \
             tc.tile_pool(name="ps1", bufs=1, space="PSUM") as ps1:
            psum_st = ps1.tile([128, Q], DT)
            for t in range(LT):
                r = 32 * (t % 3)
                cv_t = cvp.tile([128, Q], DT)
                nc.sync.dma_start(out=cv_t,
                                  in_=cv_ext[t * 128:(t + 1) * 128, :])
                nc.vector.scalar_tensor_tensor(
                    out=dummy.broadcast_to([128, Q]),
                    in0=cv_t, scalar=1.0, in1=queryB,
                    op0=mybir.AluOpType.mult, op1=mybir.AluOpType.mult,
                    accum_out=scores[:, t:t + 1],
                )
                cvb_t = cvb.tile([128, Q], BF)
                if t % 2 == 0:
                    nc.vector.tensor_copy(cvb_t, cv_t)
                else:
                    nc.scalar.copy(cvb_t, cv_t)
                nc.gpsimd.partition_all_reduce(
                    mstack[:, t:t + 1], scores[:, t:t + 1], 128, ReduceOp.max)
                nc.gpsimd.tensor_scalar_mul(
                    nstack[:, t:t + 1], mstack[:, t:t + 1], -1.0)
                nc.scalar.activation(
                    out=estack[:, t:t + 1], in_=scores[:, t:t + 1],
                    func=mybir.ActivationFunctionType.Exp,
                    bias=nstack[:, t:t + 1], scale=1.0)
                nc.gpsimd.partition_all_reduce(
                    sstack[:, t:t + 1], estack[:, t:t + 1], 128, ReduceOp.add)
                for n in range(NB):
                    sl = slice(n * 512, (n + 1) * 512)
                    nc.tensor.matmul(
                        psum_st[r:r + 1, sl],
                        lhsT=estack[:, t:t + 1],
                        rhs=cvb_t[:, sl],
                        start=True, stop=True,
                    )
                nc.scalar.copy(stage[r:r + 1, :], psum_st[r:r + 1, :])
                row_out = bass.AP(tensor=cc_in.ap().tensor, offset=t * CCW,
                                  ap=[[0, 1], [1, Q]])
                nc.scalar.dma_start(out=row_out, in_=stage[r:r + 1, :])

        m_out = bass.AP(tensor=cc_in.ap().tensor, offset=Q,
                        ap=[[0, 1], [CCW, LT]])
        nc.scalar.dma_start(out=m_out, in_=mstack[0:1, 0:LT])
        s_out = bass.AP(tensor=cc_in.ap().tensor, offset=Q + 1,
                        ap=[[0, 1], [CCW, LT]])
        nc.scalar.dma_start(out=s_out, in_=sstack[0:1, 0:LT])

        # ---- phase 2: K_w query-half dots (overlap the collective) ----
        accq = smalls.tile([128, RT], DT)
        accs = smalls.tile([128, RT], DT)
        acc = smalls.tile([128, RT], DT)
        with tc.tile_pool(name="kwq", bufs=2) as kwq:
            for j in range(RT):
                kwq_j = kwq.tile([128, Q], DT)
                nc.sync.dma_start(out=kwq_j,
                                  in_=kw_ext[j * 128:(j + 1) * 128, 0:Q])
                nc.vector.scalar_tensor_tensor(
                    out=dummy.broadcast_to([128, Q]),
                    in0=kwq_j, scalar=1.0, in1=queryB,
                    op0=mybir.AluOpType.mult, op1=mybir.AluOpType.mult,
                    accum_out=accq[:, j:j + 1],
                )

        # ---- phase 3: AllGather the 64 rows ----
        nc.gpsimd.collective_compute(
            "AllGather",
            mybir.AluOpType.bypass,
            replica_groups=[list(range(N_CORES))],
            ins=[cc_in.ap().opt()],
            outs=[cc_out.ap().opt()],
        )
        gathered = persist.tile([GROWS, CCW], DT)
        gin = bass.AP(tensor=cc_out.ap().tensor, offset=0,
                      ap=[[CCW, GROWS], [1, CCW]])
        nc.scalar.dma_start(out=gathered, in_=gin)

        # ---- phase 4: global softmax combine, s_t broadcast into PSUM ----
        mg = gathered[:, Q:Q + 1]
        sg = gathered[:, Q + 1:Q + 2]
        mmax = smalls.tile([GROWS, 1], DT)
        nc.gpsimd.partition_all_reduce(mmax, mg, GROWS, ReduceOp.max)
        negM = smalls.tile([GROWS, 1], DT)
        nc.gpsimd.tensor_scalar_mul(negM, mmax, -1.0)
        expm = smalls.tile([GROWS, 1], DT)
        nc.scalar.activation(out=expm, in_=mg,
                             func=mybir.ActivationFunctionType.Exp,
                             bias=negM, scale=1.0)
        w = smalls.tile([GROWS, 1], DT)
        nc.vector.tensor_mul(w, expm, sg)
        wsum = smalls.tile([GROWS, 1], DT)
        nc.gpsimd.partition_all_reduce(wsum, w, GROWS, ReduceOp.add)
        rS = smalls.tile([GROWS, 1], DT)
        nc.vector.reciprocal(rS, wsum)
        alpha = smalls.tile([GROWS, 1], DT)
        nc.vector.tensor_mul(alpha, expm, rS)
        alpha_rep = smalls.tile([GROWS, 128], BF)
        nc.vector.tensor_scalar_mul(alpha_rep, ones_rep, alpha)
        gathb = persist.tile([GROWS, Q], BF)
        nc.scalar.copy(gathb, gathered[:, 0:Q])

        with tc.tile_pool(name="ps2", bufs=1, space="PSUM") as ps2, \
             tc.tile_pool(name="kws", bufs=3) as kws:
            psum_stB = ps2.tile([128, Q], DT)
            for n in range(NB):
                sl = slice(n * 512, (n + 1) * 512)
                nc.tensor.matmul(
                    psum_stB[:, sl],
                    lhsT=alpha_rep,
                    rhs=gathb[0:GROWS, sl],
                    start=True, stop=True,
                )

            # ---- phase 5: K_w s_t-half dots against PSUM-resident s_t ----
            for j in range(RT):
                kws_j = kws.tile([128, Q], DT)
                nc.sync.dma_start(out=kws_j,
                                  in_=kw_ext[j * 128:(j + 1) * 128, Q:2 * Q])
                nc.vector.scalar_tensor_tensor(
                    out=dummy.broadcast_to([128, Q]),
                    in0=kws_j, scalar=1.0, in1=psum_stB,
                    op0=mybir.AluOpType.mult, op1=mybir.AluOpType.mult,
                    accum_out=accs[:, j:j + 1],
                )

        nc.vector.tensor_add(acc, accq, accs)
        nc.sync.dma_start(out=out_ext.ap(), in_=acc)

        if _DEBUG:
            nc.sync.dma_start(out=dbg_ext[0:1, 0:LT], in_=mstack[0:1, 0:LT])
            nc.sync.dma_start(out=dbg_ext[1:2, 0:LT], in_=sstack[0:1, 0:LT])
            nc.sync.dma_start(out=dbg_ext[2:3, 0:8],
                              in_=gathered[0:1, Q:Q + 8])
            nc.sync.dma_start(out=dbg_ext[3:4, 0:8], in_=scores[0:1, 0:LT])
            nc.sync.dma_start(out=dbg_ext[4:5, 0:1], in_=alpha[0:1, 0:1])
            nc.sync.dma_start(out=dbg_ext[5:6, 0:1], in_=wsum[0:1, 0:1])
            nc.sync.dma_start(out=dbg_ext[6:7, 0:4], in_=accq[0:1, 0:4])
            nc.sync.dma_start(out=dbg_ext[7:8, 0:4], in_=accs[0:1, 0:4])
            nc.sync.dma_start(out=dbg_ext[8:9, 0:4], in_=gathered[0:1, 0:4])

    nc.compile()
    return nc


def get_nc():
    if "nc" not in _NC_CACHE:
        _NC_CACHE["nc"] = build_nc()
    return _NC_CACHE["nc"]


def _shard_inputs(query, context_vector, K_w):
    q2 = np.ascontiguousarray(query.reshape(1, Q), dtype=np.float32)
    in_maps = []
    for c in range(N_CORES):
        in_maps.append({
            "query": q2,
            "cv": np.ascontiguousarray(
                context_vector[c * L_SHARD:(c + 1) * L_SHARD], dtype=np.float32),
            "kw": np.ascontiguousarray(
                K_w[c * R_SHARD:(c + 1) * R_SHARD], dtype=np.float32),
        })
    return in_maps


def kernel(query, context_vector, K_w, _trace=False, _trace_kwargs=None):
    nc = get_nc()
    in_maps = _shard_inputs(query, context_vector, K_w)
    res = run_bass_kernel_spmd(nc, in_maps, core_ids=list(range(N_CORES)),
                               trace=_trace, **(_trace_kwargs or {}))
    out = np.concatenate(
        [np.asarray(res.results[c]["out"]).T.reshape(-1) for c in range(N_CORES)]
    ).astype(np.float32)
    if _trace:
        kernel.last_results = res
    return out


# revision 11
# speedup vs baseline: 1.2752x; 1.0470x over previous
"""Distributed Trainium2 kernel for the attention GEMV chain:

    score = context_vector @ query            [L]         (L=8192, Q=4096)
    attn  = softmax(score)
    s_t   = attn @ context_vector             [Q]
    out   = K_w @ concat(query, s_t)          [Q]

Sharding over 8 NeuronCores:
  - context_vector rows: 1024 per core (score GEMV + partial weighted sums)
  - K_w rows: 512 per core (each core produces its own slice of the final
    output, so no output collective is needed)
  - per-core flash-softmax partials are combined on-chip into ONE row, so
    the AllGather moves only [s_t_local(4096), ref_max, expsum] per core;
    the global normalization finishes after the gather with an
    alpha-weighted rank-8 bf16 matmul that also broadcasts s_t to all 128
    partitions (in PSUM).

Per-core schedule (flash-style, paced by the DMA stream):
  - per 128-row cv tile: fused mult+reduce (scalar_tensor_tensor) gives the
    128 scores in one DVE pass; per GROUP of 3 tiles one gpsimd
    partition_all_reduce gives the tile maxes (per-tile reference keeps
    exp <= 1, always fp32-safe); ACT computes bf16 exp weights; TensorE
    matmuls the exp-weighted row sum into a PSUM row cycling partitions
    {0,32,64} with cheap bf16 matmuls; ACT copies each row out and a
    casting SWDGE DMA lands it on its own partition for the local combine.
  - scores stay fp32 end-to-end (softmax is argmax-dominated); the weights
    and matrices tolerate bf16 (verified ~1e-3 rel err).
  - K_w streams as 8 half-tiles [128, 4096]; the query-half dot products
    run before/during the collective, the s_t-half ones read the broadcast
    s_t directly from PSUM.
"""
import sys

if "/opt/trn_rl_repo" not in sys.path:
    sys.path.insert(0, "/opt/trn_rl_repo")

from contextlib import ExitStack

import numpy as np

import concourse.bass as bass
import concourse.bacc as bacc
import concourse.mybir as mybir
import concourse.tile as tile
from concourse.bass_isa import ReduceOp
from concourse.bass_utils import run_bass_kernel_spmd

N_CORES = 8
Q = 4096
L = 8192
L_SHARD = L // N_CORES          # 1024 rows of context_vector per core
R_SHARD = Q // N_CORES          # 512 rows of K_w per core
LT = L_SHARD // 128             # 8 l-tiles per core
RT = R_SHARD // 128             # 4 r-tiles per core
NB = Q // 512                   # 8 psum banks of 512 fp32
CCW = Q + 8                     # collective row: s_t_local, max, sum, pad
GROUPS = [(0, 3), (3, 6), (6, 8)]   # tile groups sharing one gpsimd max op
DT = mybir.dt.float32
BF = mybir.dt.bfloat16

_NC_CACHE = {}
_DEBUG = False


def build_nc():
    nc = bacc.Bacc("TRN2", target_bir_lowering=False, debug=False,
                   num_devices=N_CORES)

    q_ext = nc.dram_tensor("query", [1, Q], DT, kind="ExternalInput")
    cv_ext = nc.dram_tensor("cv", [L_SHARD, Q], DT, kind="ExternalInput")
    kw_ext = nc.dram_tensor("kw", [R_SHARD, 2 * Q], DT, kind="ExternalInput")
    out_ext = nc.dram_tensor("out", [128, RT], DT, kind="ExternalOutput")

    cc_in = nc.dram_tensor("cc_in", [1, CCW], DT)
    cc_out = nc.dram_tensor("cc_out", [N_CORES, CCW], DT, addr_space="Shared")
    dbg_ext = None
    if _DEBUG:
        dbg_ext = nc.dram_tensor("dbg", [16, 16], DT, kind="ExternalOutput")

    with tile.TileContext(nc) as tc, ExitStack() as ctx:
        persist = ctx.enter_context(tc.tile_pool(name="persist", bufs=1))
        smalls = ctx.enter_context(tc.tile_pool(name="smalls", bufs=1))

        # query to SBUF, broadcast to all 128 partitions on gpsimd
        q_sb = persist.tile([1, Q], DT)
        nc.sync.dma_start(out=q_sb, in_=q_ext.ap())
        queryB = persist.tile([128, Q], DT)
        nc.gpsimd.partition_broadcast(queryB, q_sb, 128)

        scores = smalls.tile([128, LT], DT)
        dummy = smalls.tile([128, 1], DT)
        mstack = smalls.tile([128, LT], DT)     # per-tile max (replicated)
        nstack = smalls.tile([128, LT], DT)     # negated maxes
        estack = smalls.tile([128, LT], BF)     # per-tile bf16 exp weights
        mvec = smalls.tile([LT, 1], DT)         # m_t placed on partition t
        stage = persist.tile([128, Q], DT)      # staged rows at {0,32,64}
        localb = persist.tile([LT, Q], BF)      # per-tile rows on partition t
        ones_rep = smalls.tile([N_CORES, 128], BF)
        nc.vector.memset(ones_rep, 1.0)

        # ---- phase 1: stream cv; per-tile scores, stats, weighted row ----
        with tc.tile_pool(name="cvp", bufs=2) as cvp, \
             tc.tile_pool(name="cvb", bufs=3) as cvb, \
             tc.tile_pool(name="ps1", bufs=1, space="PSUM") as ps1:
            psum_st = ps1.tile([128, Q], DT)
            cvb_tiles = {}
            for g0, g1 in GROUPS:
                for t in range(g0, g1):
                    cv_t = cvp.tile([128, Q], DT)
                    nc.sync.dma_start(out=cv_t,
                                      in_=cv_ext[t * 128:(t + 1) * 128, :])
                    nc.vector.scalar_tensor_tensor(
                        out=dummy.broadcast_to([128, Q]),
                        in0=cv_t, scalar=1.0, in1=queryB,
                        op0=mybir.AluOpType.mult, op1=mybir.AluOpType.mult,
                        accum_out=scores[:, t:t + 1],
                    )
                    cvb_t = cvb.tile([128, Q], BF)
                    if t % 2 == 0:
                        nc.vector.tensor_copy(cvb_t, cv_t)
                    else:
                        nc.scalar.copy(cvb_t, cv_t)
                    cvb_tiles[t] = cvb_t
                # one cross-partition max per group
                nc.gpsimd.partition_all_reduce(
                    mstack[:, g0:g1], scores[:, g0:g1], 128, ReduceOp.max)
                nc.vector.tensor_scalar_mul(
                    nstack[:, g0:g1], mstack[:, g0:g1], -1.0)
                for t in range(g0, g1):
                    r = 32 * (t % 3)
                    nc.scalar.activation(
                        out=estack[:, t:t + 1], in_=scores[:, t:t + 1],
                        func=mybir.ActivationFunctionType.Exp,
                        bias=nstack[:, t:t + 1], scale=1.0)
                    for n in range(NB):
                        sl = slice(n * 512, (n + 1) * 512)
                        nc.tensor.matmul(
                            psum_st[r:r + 1, sl],
                            lhsT=estack[:, t:t + 1],
                            rhs=cvb_tiles[t][:, sl],
                            start=True, stop=True,
                        )
                    del cvb_tiles[t]
                    nc.scalar.copy(stage[r:r + 1, :], psum_st[r:r + 1, :])
                    # land the row on partition t, casting to bf16 (SWDGE)
                    nc.gpsimd.dma_start(out=localb[t:t + 1, :],
                                        in_=stage[r:r + 1, :])
                    # place m_t on partition t for the local-combine weights
                    nc.scalar.dma_start(out=mvec[t:t + 1, 0:1],
                                        in_=mstack[0:1, t:t + 1])

            # ---- local softmax combine across the core's 8 tiles ----
            sstack = smalls.tile([128, LT], DT)
            nc.gpsimd.partition_all_reduce(sstack, estack, 128, ReduceOp.add)
            Mc = smalls.tile([128, 1], DT)
            nc.vector.tensor_reduce(out=Mc, in_=mstack,
                                    axis=mybir.AxisListType.X,
                                    op=mybir.AluOpType.max)
            negMc = smalls.tile([128, 1], DT)
            nc.vector.tensor_scalar_mul(negMc, Mc, -1.0)
            beta_full = smalls.tile([128, LT], DT)
            nc.scalar.activation(out=beta_full, in_=mstack,
                                 func=mybir.ActivationFunctionType.Exp,
                                 bias=negMc, scale=1.0)
            Sc = smalls.tile([128, 1], DT)
            nc.vector.scalar_tensor_tensor(
                out=dummy.broadcast_to([128, LT]),
                in0=sstack, scalar=1.0, in1=beta_full,
                op0=mybir.AluOpType.mult, op1=mybir.AluOpType.mult,
                accum_out=Sc,
            )
            beta_vec = smalls.tile([LT, 1], BF)
            nc.scalar.activation(out=beta_vec, in_=mvec,
                                 func=mybir.ActivationFunctionType.Exp,
                                 bias=negMc[0:LT, 0:1], scale=1.0)
            # combine the 8 rows: psum row 0 <- sum_t beta_t * localb[t]
            for n in range(NB):
                sl = slice(n * 512, (n + 1) * 512)
                nc.tensor.matmul(
                    psum_st[0:1, sl],
                    lhsT=beta_vec,
                    rhs=localb[0:LT, sl],
                    start=True, stop=True,
                )
            st_row = persist.tile([1, Q], DT)
            nc.scalar.copy(st_row, psum_st[0:1, :])

        nc.scalar.dma_start(out=cc_in[0:1, 0:Q], in_=st_row)
        nc.scalar.dma_start(out=cc_in[0:1, Q:Q + 1], in_=Mc[0:1, 0:1])
        nc.scalar.dma_start(out=cc_in[0:1, Q + 1:Q + 2], in_=Sc[0:1, 0:1])

        # ---- phase 2: K_w query-half dots (overlap the collective) ----
        accq = smalls.tile([128, RT], DT)
        accs = smalls.tile([128, RT], DT)
        acc = smalls.tile([128, RT], DT)
        with tc.tile_pool(name="kwq", bufs=2) as kwq:
            for j in range(RT):
                kwq_j = kwq.tile([128, Q], DT)
                nc.sync.dma_start(out=kwq_j,
                                  in_=kw_ext[j * 128:(j + 1) * 128, 0:Q])
                nc.vector.scalar_tensor_tensor(
                    out=dummy.broadcast_to([128, Q]),
                    in0=kwq_j, scalar=1.0, in1=queryB,
                    op0=mybir.AluOpType.mult, op1=mybir.AluOpType.mult,
                    accum_out=accq[:, j:j + 1],
                )

        # ---- phase 3: AllGather of [s_t_local | ref | expsum] ----
        nc.gpsimd.collective_compute(
            "AllGather",
            mybir.AluOpType.bypass,
            replica_groups=[list(range(N_CORES))],
            ins=[cc_in.ap().opt()],
            outs=[cc_out.ap().opt()],
        )
        gathered = persist.tile([N_CORES, CCW], DT)
        nc.scalar.dma_start(out=gathered, in_=cc_out.ap())

        # ---- phase 4: global softmax combine, s_t broadcast into PSUM ----
        mg = gathered[:, Q:Q + 1]
        sg = gathered[:, Q + 1:Q + 2]
        mmax = smalls.tile([N_CORES, 1], DT)
        nc.gpsimd.partition_all_reduce(mmax, mg, N_CORES, ReduceOp.max)
        negM = smalls.tile([N_CORES, 1], DT)
        nc.vector.tensor_scalar_mul(negM, mmax, -1.0)
        expm = smalls.tile([N_CORES, 1], DT)
        nc.scalar.activation(out=expm, in_=mg,
                             func=mybir.ActivationFunctionType.Exp,
                             bias=negM, scale=1.0)
        w = smalls.tile([N_CORES, 1], DT)
        nc.vector.tensor_mul(w, expm, sg)
        wsum = smalls.tile([N_CORES, 1], DT)
        nc.gpsimd.partition_all_reduce(wsum, w, N_CORES, ReduceOp.add)
        rS = smalls.tile([N_CORES, 1], DT)
        nc.vector.reciprocal(rS, wsum)
        alpha = smalls.tile([N_CORES, 1], DT)
        nc.vector.tensor_mul(alpha, expm, rS)
        alpha_rep = smalls.tile([N_CORES, 128], BF)
        nc.vector.tensor_scalar_mul(alpha_rep, ones_rep, alpha)
        gathb = persist.tile([N_CORES, Q], BF)
        nc.scalar.copy(gathb, gathered[:, 0:Q])

        with tc.tile_pool(name="ps2", bufs=1, space="PSUM") as ps2, \
             tc.tile_pool(name="kws", bufs=2) as kws:
            psum_stB = ps2.tile([128, Q], DT)
            for n in range(NB):
                sl = slice(n * 512, (n + 1) * 512)
                nc.tensor.matmul(
                    psum_stB[:, sl],
                    lhsT=alpha_rep,
                    rhs=gathb[0:N_CORES, sl],
                    start=True, stop=True,
                )

            # ---- phase 5: K_w s_t-half dots against PSUM-resident s_t ----
            for j in range(RT):
                kws_j = kws.tile([128, Q], DT)
                nc.sync.dma_start(out=kws_j,
                                  in_=kw_ext[j * 128:(j + 1) * 128, Q:2 * Q])
                nc.vector.scalar_tensor_tensor(
                    out=dummy.broadcast_to([128, Q]),
                    in0=kws_j, scalar=1.0, in1=psum_stB,
                    op0=mybir.AluOpType.mult, op1=mybir.AluOpType.mult,
                    accum_out=accs[:, j:j + 1],
                )

        nc.vector.tensor_add(acc, accq, accs)
        nc.sync.dma_start(out=out_ext.ap(), in_=acc)

        if _DEBUG:
            nc.sync.dma_start(out=dbg_ext[0:1, 0:LT], in_=mstack[0:1, 0:LT])
            nc.sync.dma_start(out=dbg_ext[1:2, 0:LT], in_=sstack[0:1, 0:LT])
            nc.sync.dma_start(out=dbg_ext[2:3, 0:8],
                              in_=gathered[0:1, Q:Q + 8])
            nc.sync.dma_start(out=dbg_ext[3:4, 0:8], in_=scores[0:1, 0:LT])
            nc.sync.dma_start(out=dbg_ext[4:5, 0:1], in_=alpha[0:1, 0:1])
            nc.sync.dma_start(out=dbg_ext[5:6, 0:1], in_=wsum[0:1, 0:1])
            nc.sync.dma_start(out=dbg_ext[6:7, 0:4], in_=accq[0:1, 0:4])
            nc.sync.dma_start(out=dbg_ext[7:8, 0:4], in_=accs[0:1, 0:4])
            nc.sync.dma_start(out=dbg_ext[8:9, 0:4], in_=st_row[0:1, 0:4])
            nc.sync.dma_start(out=dbg_ext[9:10, 0:LT], in_=mvec[0:LT, 0:1])

    nc.compile()
    return nc


def get_nc():
    if "nc" not in _NC_CACHE:
        _NC_CACHE["nc"] = build_nc()
    return _NC_CACHE["nc"]


def _shard_inputs(query, context_vector, K_w):
    q2 = np.ascontiguousarray(query.reshape(1, Q), dtype=np.float32)
    in_maps = []
    for c in range(N_CORES):
        in_maps.append({
            "query": q2,
            "cv": np.ascontiguousarray(
                context_vector[c * L_SHARD:(c + 1) * L_SHARD], dtype=np.float32),
            "kw": np.ascontiguousarray(
                K_w[c * R_SHARD:(c + 1) * R_SHARD], dtype=np.float32),
        })
    return in_maps


def kernel(query, context_vector, K_w, _trace=False, _trace_kwargs=None):
    nc = get_nc()
    in_maps = _shard_inputs(query, context_vector, K_w)
    res = run_bass_kernel_spmd(nc, in_maps, core_ids=list(range(N_CORES)),
                               trace=_trace, **(_trace_kwargs or {}))
    out = np.concatenate(
        [np.asarray(res.results[c]["out"]).T.reshape(-1) for c in range(N_CORES)]
    ).astype(np.float32)
    if _trace:
        kernel.last_results = res
    return out


# revision 15
# speedup vs baseline: 1.4195x; 1.1131x over previous
"""Distributed Trainium2 kernel for the attention GEMV chain:

    score = context_vector @ query            [L]         (L=8192, Q=4096)
    attn  = softmax(score)
    s_t   = attn @ context_vector             [Q]
    out   = K_w @ concat(query, s_t)          [Q]

Sharding over 8 NeuronCores:
  - context_vector rows: 1024 per core (score GEMV + partial weighted sums)
  - K_w rows: 512 per core (each core produces its own slice of the final
    output, so no output collective is needed)
  - flash-softmax partials are accumulated per GROUP of 3 cv tiles (group
    max as the exp reference keeps exp <= 1, always fp32-safe); one
    AllGather moves the 3 group rows [s_t_grp(4096), grp_max, grp_expsum]
    per core; the global normalization finishes after the gather with an
    alpha-weighted rank-24 bf16 matmul that also broadcasts s_t to all 128
    partitions (in PSUM).

Per-core schedule (paced by the DMA stream):
  - query arrives pre-broadcast [128, 4096] from the host shard prep (a
    stride-0 broadcast DMA measures ~3x slower than a plain 2MB load).
  - per 128-row cv tile: fused mult+reduce (scalar_tensor_tensor) gives the
    128 scores in one DVE pass; the tile is cast to bf16 (alternating
    DVE/ACT to balance load); per group one gpsimd partition_all_reduce
    gives the maxes; ACT computes bf16 exp weights; TensorE accumulates the
    exp-weighted rows into the group's PSUM row (partitions {0,32,64})
    with cheap bf16 matmuls; one ACT copy per group stages the row.
  - scores stay fp32 end-to-end (softmax is argmax-dominated); the weights
    and matrices tolerate bf16 (verified ~1e-3 rel err).
  - K_w streams as 8 half-tiles [128, 4096]; the query-half dot products
    run before/during the collective, the s_t-half ones read the broadcast
    s_t directly from PSUM.
"""
import sys

if "/opt/trn_rl_repo" not in sys.path:
    sys.path.insert(0, "/opt/trn_rl_repo")

from contextlib import ExitStack

import numpy as np

import concourse.bass as bass
import concourse.bacc as bacc
import concourse.mybir as mybir
import concourse.tile as tile
from concourse.bass_isa import ReduceOp
from concourse.bass_utils import run_bass_kernel_spmd

N_CORES = 8
Q = 4096
L = 8192
L_SHARD = L // N_CORES          # 1024 rows of context_vector per core
R_SHARD = Q // N_CORES          # 512 rows of K_w per core
LT = L_SHARD // 128             # 8 l-tiles per core
RT = R_SHARD // 128             # 4 r-tiles per core
NB = Q // 512                   # 8 psum banks of 512 fp32
CCW = Q + 8                     # collective row: s_t_grp, max, sum, pad
GROUPS = [(0, 3), (3, 6), (6, 8)]   # cv tile groups, one PSUM row each
NG = len(GROUPS)
GROWS = N_CORES * NG            # 24 gathered rows
DT = mybir.dt.float32
BF = mybir.dt.bfloat16

_NC_CACHE = {}
_DEBUG = False


def build_nc():
    nc = bacc.Bacc("TRN2", target_bir_lowering=False, debug=False,
                   num_devices=N_CORES)

    q_ext = nc.dram_tensor("query", [128, Q], DT, kind="ExternalInput")
    cv_ext = nc.dram_tensor("cv", [L_SHARD, Q], DT, kind="ExternalInput")
    kw_ext = nc.dram_tensor("kw", [R_SHARD, 2 * Q], DT, kind="ExternalInput")
    out_ext = nc.dram_tensor("out", [128, RT], DT, kind="ExternalOutput")

    cc_in = nc.dram_tensor("cc_in", [1, NG * CCW], DT)
    cc_out = nc.dram_tensor("cc_out", [N_CORES, NG * CCW], DT,
                            addr_space="Shared")
    dbg_ext = None
    if _DEBUG:
        dbg_ext = nc.dram_tensor("dbg", [16, 16], DT, kind="ExternalOutput")

    with tile.TileContext(nc) as tc, ExitStack() as ctx:
        persist = ctx.enter_context(tc.tile_pool(name="persist", bufs=1))
        smalls = ctx.enter_context(tc.tile_pool(name="smalls", bufs=1))

        queryB = persist.tile([128, Q], DT)
        nc.sync.dma_start(out=queryB, in_=q_ext.ap())

        scores = smalls.tile([128, LT], DT)
        dummy = smalls.tile([128, 1], DT)
        mstack = smalls.tile([128, NG], DT)     # per-group max (replicated)
        nstack = smalls.tile([128, NG], DT)     # negated maxes
        estack = smalls.tile([128, LT], BF)     # per-tile bf16 exp weights
        sgrp = smalls.tile([128, NG], DT)       # per-group expsum
        stage = persist.tile([128, Q], DT)      # staged rows at {0,32,64}
        tmp_max = smalls.tile([128, LT], DT)    # per-column partition maxes
        ones_rep = smalls.tile([GROWS, 128], BF)
        nc.vector.memset(ones_rep, 1.0)

        # ---- phase 1: stream cv; per-group scores, stats, weighted row ----
        with tc.tile_pool(name="cvp", bufs=3) as cvp, \
             tc.tile_pool(name="cvb", bufs=4) as cvb, \
             tc.tile_pool(name="ps1", bufs=1, space="PSUM") as ps1:
            psum_st = ps1.tile([128, Q], DT)
            for g, (g0, g1) in enumerate(GROUPS):
                r = 32 * g
                gsz = g1 - g0
                cvb_tiles = {}
                for t in range(g0, g1):
                    cv_t = cvp.tile([128, Q], DT)
                    nc.sync.dma_start(out=cv_t,
                                      in_=cv_ext[t * 128:(t + 1) * 128, :])
                    nc.vector.scalar_tensor_tensor(
                        out=dummy.broadcast_to([128, Q]),
                        in0=cv_t, scalar=1.0, in1=queryB,
                        op0=mybir.AluOpType.mult, op1=mybir.AluOpType.mult,
                        accum_out=scores[:, t:t + 1],
                    )
                    cvb_t = cvb.tile([128, Q], BF)
                    if t % 2 == 0:
                        nc.vector.tensor_copy(cvb_t, cv_t)
                    else:
                        nc.scalar.copy(cvb_t, cv_t)
                    cvb_tiles[t] = cvb_t
                # group stats: cross-partition max per column, then one
                # group max on DVE, negate, per-tile exp weights
                nc.gpsimd.partition_all_reduce(
                    tmp_max[:, g0:g1], scores[:, g0:g1], 128, ReduceOp.max)
                nc.vector.tensor_reduce(
                    out=mstack[:, g:g + 1], in_=tmp_max[:, g0:g1],
                    axis=mybir.AxisListType.X, op=mybir.AluOpType.max)
                nc.vector.tensor_scalar_mul(
                    nstack[:, g:g + 1], mstack[:, g:g + 1], -1.0)
                for t in range(g0, g1):
                    nc.scalar.activation(
                        out=estack[:, t:t + 1], in_=scores[:, t:t + 1],
                        func=mybir.ActivationFunctionType.Exp,
                        bias=nstack[:, g:g + 1], scale=1.0)
                    for n in range(NB):
                        sl = slice(n * 512, (n + 1) * 512)
                        nc.tensor.matmul(
                            psum_st[r:r + 1, sl],
                            lhsT=estack[:, t:t + 1],
                            rhs=cvb_tiles[t][:, sl],
                            start=(t == g0), stop=(t == g1 - 1),
                            skip_group_check=True,
                        )
                # group expsum and row staging + collective-buffer shipping
                se = smalls.tile([128, 1], DT)
                nc.vector.tensor_reduce(
                    out=se, in_=estack[:, g0:g1],
                    axis=mybir.AxisListType.X, op=mybir.AluOpType.add)
                nc.gpsimd.partition_all_reduce(
                    sgrp[:, g:g + 1], se, 128, ReduceOp.add)
                nc.scalar.copy(stage[r:r + 1, :], psum_st[r:r + 1, :])
                row_out = bass.AP(tensor=cc_in.ap().tensor, offset=g * CCW,
                                  ap=[[0, 1], [1, Q]])
                nc.scalar.dma_start(out=row_out, in_=stage[r:r + 1, :])
                nc.scalar.dma_start(
                    out=cc_in[0:1, g * CCW + Q:g * CCW + Q + 1],
                    in_=mstack[0:1, g:g + 1])
                nc.scalar.dma_start(
                    out=cc_in[0:1, g * CCW + Q + 1:g * CCW + Q + 2],
                    in_=sgrp[0:1, g:g + 1])

        # ---- phase 2: K_w query-half dots (overlap the collective) ----
        accq = smalls.tile([128, RT], DT)
        accs = smalls.tile([128, RT], DT)
        acc = smalls.tile([128, RT], DT)
        with tc.tile_pool(name="kwq", bufs=2) as kwq:
            for j in range(RT):
                kwq_j = kwq.tile([128, Q], DT)
                nc.sync.dma_start(out=kwq_j,
                                  in_=kw_ext[j * 128:(j + 1) * 128, 0:Q])
                nc.vector.scalar_tensor_tensor(
                    out=dummy.broadcast_to([128, Q]),
                    in0=kwq_j, scalar=1.0, in1=queryB,
                    op0=mybir.AluOpType.mult, op1=mybir.AluOpType.mult,
                    accum_out=accq[:, j:j + 1],
                )

        # ---- phase 3: AllGather of the 24 group rows ----
        nc.gpsimd.collective_compute(
            "AllGather",
            mybir.AluOpType.bypass,
            replica_groups=[list(range(N_CORES))],
            ins=[cc_in.ap().opt()],
            outs=[cc_out.ap().opt()],
        )

        late = ctx.enter_context(tc.tile_pool(name="late", bufs=1))
        gathered = late.tile([GROWS, CCW], DT)
        gin = bass.AP(tensor=cc_out.ap().tensor, offset=0,
                      ap=[[CCW, GROWS], [1, CCW]])
        nc.scalar.dma_start(out=gathered, in_=gin)

        # ---- phase 4: global softmax combine, s_t broadcast into PSUM ----
        mg = gathered[:, Q:Q + 1]
        sg = gathered[:, Q + 1:Q + 2]
        mmax = smalls.tile([GROWS, 1], DT)
        nc.gpsimd.partition_all_reduce(mmax, mg, GROWS, ReduceOp.max)
        negM = smalls.tile([GROWS, 1], DT)
        nc.vector.tensor_scalar_mul(negM, mmax, -1.0)
        expm = smalls.tile([GROWS, 1], DT)
        nc.scalar.activation(out=expm, in_=mg,
                             func=mybir.ActivationFunctionType.Exp,
                             bias=negM, scale=1.0)
        w = smalls.tile([GROWS, 1], DT)
        nc.vector.tensor_mul(w, expm, sg)
        wsum = smalls.tile([GROWS, 1], DT)
        nc.gpsimd.partition_all_reduce(wsum, w, GROWS, ReduceOp.add)
        rS = smalls.tile([GROWS, 1], DT)
        nc.vector.reciprocal(rS, wsum)
        alpha = smalls.tile([GROWS, 1], DT)
        nc.vector.tensor_mul(alpha, expm, rS)
        alpha_rep = smalls.tile([GROWS, 128], BF)
        nc.vector.tensor_scalar_mul(alpha_rep, ones_rep, alpha)
        gathb = late.tile([GROWS, Q], BF)
        nc.scalar.copy(gathb, gathered[:, 0:Q])

        with tc.tile_pool(name="ps2", bufs=1, space="PSUM") as ps2, \
             tc.tile_pool(name="kws", bufs=4) as kws:
            psum_stB = ps2.tile([128, Q], DT)
            for n in range(NB):
                sl = slice(n * 512, (n + 1) * 512)
                nc.tensor.matmul(
                    psum_stB[:, sl],
                    lhsT=alpha_rep,
                    rhs=gathb[0:GROWS, sl],
                    start=True, stop=True,
                )

            # ---- phase 5: K_w s_t-half dots against PSUM-resident s_t ----
            for j in range(RT):
                kws_j = kws.tile([128, Q], DT)
                nc.sync.dma_start(out=kws_j,
                                  in_=kw_ext[j * 128:(j + 1) * 128, Q:2 * Q])
                nc.vector.scalar_tensor_tensor(
                    out=dummy.broadcast_to([128, Q]),
                    in0=kws_j, scalar=1.0, in1=psum_stB,
                    op0=mybir.AluOpType.mult, op1=mybir.AluOpType.mult,
                    accum_out=accs[:, j:j + 1],
                )

        nc.vector.tensor_add(acc, accq, accs)
        nc.sync.dma_start(out=out_ext.ap(), in_=acc)

        if _DEBUG:
            nc.sync.dma_start(out=dbg_ext[0:1, 0:NG], in_=mstack[0:1, 0:NG])
            nc.sync.dma_start(out=dbg_ext[1:2, 0:NG], in_=sgrp[0:1, 0:NG])
            nc.sync.dma_start(out=dbg_ext[2:3, 0:8],
                              in_=gathered[0:1, Q:Q + 8])
            nc.sync.dma_start(out=dbg_ext[3:4, 0:8], in_=scores[0:1, 0:LT])
            nc.sync.dma_start(out=dbg_ext[4:5, 0:1], in_=alpha[0:1, 0:1])
            nc.sync.dma_start(out=dbg_ext[5:6, 0:1], in_=wsum[0:1, 0:1])
            nc.sync.dma_start(out=dbg_ext[6:7, 0:4], in_=accq[0:1, 0:4])
            nc.sync.dma_start(out=dbg_ext[7:8, 0:4], in_=accs[0:1, 0:4])
            nc.sync.dma_start(out=dbg_ext[8:9, 0:4], in_=stage[0:1, 0:4])

    nc.compile()
    return nc


def get_nc():
    if "nc" not in _NC_CACHE:
        _NC_CACHE["nc"] = build_nc()
    return _NC_CACHE["nc"]


def _shard_inputs(query, context_vector, K_w):
    qb = np.ascontiguousarray(
        np.broadcast_to(np.asarray(query, dtype=np.float32).reshape(1, Q),
                        (128, Q)))
    in_maps = []
    for c in range(N_CORES):
        in_maps.append({
            "query": qb,
            "cv": np.ascontiguousarray(
                context_vector[c * L_SHARD:(c + 1) * L_SHARD], dtype=np.float32),
            "kw": np.ascontiguousarray(
                K_w[c * R_SHARD:(c + 1) * R_SHARD], dtype=np.float32),
        })
    return in_maps


def kernel(query, context_vector, K_w, _trace=False, _trace_kwargs=None):
    nc = get_nc()
    in_maps = _shard_inputs(query, context_vector, K_w)
    res = run_bass_kernel_spmd(nc, in_maps, core_ids=list(range(N_CORES)),
                               trace=_trace, **(_trace_kwargs or {}))
    out = np.concatenate(
        [np.asarray(res.results[c]["out"]).T.reshape(-1) for c in range(N_CORES)]
    ).astype(np.float32)
    if _trace:
        kernel.last_results = res
    return out
